# revision 1
# baseline (speedup 1.0000x reference)
"""GQA kernel for Trainium2, 8 NeuronCores.

Problem: x[1,4096,2048], H=16 heads, G=4 kv-groups, D=128, causal mask,
RoPE on q/k, out-proj. Sharding: 2 heads per core (core c -> heads 2c,2c+1,
kv-group c//2). All tensors are fed to the device pre-transposed so every
matmul contraction dim lands on SBUF partitions:

  phase 1: QT/KT/V projections from xT (streamed once) + RoPE
  phase 2: per-head causal attention in scoresT orientation:
           scoresT[k,q] tiles -> exp (ACT, scale=1/sqrt(D)) -> causal mask by
           affine_select -> ctxT accum on PE; softmax denom = ones-matmul over
           a DVE-accumulated exp-sum tile; normalize via PE broadcast matmul.
  phase 3: AllGather ctxT (4MB/core) then column-parallel out-proj.

Output per core: outT_c = out.T[c*256:(c+1)*256, :]; host concatenates and
transposes back.
"""

import sys

for _p in ("/opt/trn_rl_repo",):
    if _p not in sys.path:
        sys.path.append(_p)

from contextlib import ExitStack

import numpy as np

import concourse.bass as bass
import concourse.tile as tile
from concourse import masks, mybir
from concourse.bass_utils import run_bass_kernel_spmd

F32 = mybir.dt.float32
S = 4096
MAX_WAITS = 1  # walrus CoreV3 rejects instructions with more sync waits


def _split_sync_waits(nc, maxw=MAX_WAITS):
    """Hoist excess sem waits onto NOPs inserted before the instruction on
    the same engine queue (queue order makes this equivalent)."""
    from concourse import mybir as mb
    n = 0
    for bassbb in nc.bb_map.values():
        bb = bassbb.bb
        insts = list(bb.instructions)
        out = []
        changed = False
        for ins in insts:
            si = ins.sync_info
            if si is not None and si.on_wait and len(si.on_wait) > maxw:
                waits = list(si.on_wait)
                head, rest = waits[:-maxw], waits[-maxw:]
                while head:
                    chunk, head = head[:maxw], head[maxw:]
                    n += 1
                    nop = mb.InstNoOp(
                        name=f"I-ws{n}",
                        engine=ins.engine,
                        ins=[],
                        outs=[],
                        sync_info=mb.SyncInfo(on_wait=chunk, on_update=[]),
                    )
                    nc.register_instruction(nop)
                    out.append(nop)
                ins.sync_info = mb.SyncInfo(
                    on_wait=rest, on_update=list(si.on_update or []))
                changed = True
            out.append(ins)
        if changed:
            try:
                bb.instructions[:] = out
            except TypeError:
                bb.set_instructions(out)
    return n
DIN = 2048
D = 128
HPC = 2          # heads per core
NCORES = 8
QC = 512         # q-chunk (free dim per matmul)
NQ = S // QC     # 8 q-chunks
KT = 128         # k tile (partition dim)
NKIN = DIN // 128  # 16 contraction tiles for projections
INV_SQRT_D = 1.0 / np.sqrt(D)


def build_nc(debug=False):
    nc = bass.Bass(num_devices=NCORES)

    xT = nc.dram_tensor("xT", [DIN, S], F32, kind="ExternalInput")
    wqT = nc.dram_tensor("wqT", [DIN, HPC * D], F32, kind="ExternalInput")
    wkT = nc.dram_tensor("wkT", [DIN, D], F32, kind="ExternalInput")
    wvT = nc.dram_tensor("wvT", [DIN, D], F32, kind="ExternalInput")
    cosT = nc.dram_tensor("cosT", [D, S], F32, kind="ExternalInput")
    sinT = nc.dram_tensor("sinT", [D, S], F32, kind="ExternalInput")
    woT = nc.dram_tensor("woT", [DIN, HPC * D], F32, kind="ExternalInput")
    outT = nc.dram_tensor("outT", [HPC * D, S], F32, kind="ExternalOutput")

    # collective bounce buffers (collectives can't touch I/O tensors;
    # input must NOT be Shared, output should be Shared)
    ctx_local = nc.dram_tensor("ctx_local", [HPC * D, S], F32)
    ctx_full = nc.dram_tensor("ctx_full", [NCORES * HPC * D, S], F32,
                              addr_space="Shared")
    if debug:
        dbg_qt = nc.dram_tensor("dbg_qt", [128, S], F32, kind="ExternalOutput")
        dbg_kt = nc.dram_tensor("dbg_kt", [128, S], F32, kind="ExternalOutput")
        dbg_vt = nc.dram_tensor("dbg_vt", [128, S // 128, D], F32,
                                kind="ExternalOutput")
        dbg_cl = nc.dram_tensor("dbg_cl", [HPC * D, S], F32,
                                kind="ExternalOutput")
        dbg_cf = nc.dram_tensor("dbg_cf", [NCORES * HPC * D, S], F32,
                                kind="ExternalOutput")

    with ExitStack() as ctx:
        tc = ctx.enter_context(tile.TileContext(nc))

        res = ctx.enter_context(tc.tile_pool(name="res", bufs=1))
        # resident SBUF tensors
        qt = res.tile([128, HPC, S], F32, tag="qt")          # QT per head
        kt = res.tile([128, S], F32, tag="kt")               # KT (shared group)
        vt = res.tile([128, S // 128, D], F32, tag="vt")     # V as s-tiles
        wq_sb = res.tile([128, NKIN, HPC * D], F32, tag="wq")
        wk_sb = res.tile([128, NKIN, D], F32, tag="wk")
        wv_sb = res.tile([128, NKIN, D], F32, tag="wv")
        wo_sb = res.tile([128, NKIN, HPC * D], F32, tag="wo")
        ones_k = res.tile([128, 1], F32, tag="ones_k")       # lhsT for col sums
        ones_r = res.tile([1, 128], F32, tag="ones_r")       # lhsT for bcast
        ident = res.tile([128, 128], F32, tag="ident")       # PE transpose id

        nc.vector.memset(ones_k, 1.0)
        nc.vector.memset(ones_r, 1.0)
        masks.make_identity(nc, ident)

        # weight loads
        nc.sync.dma_start(out=wq_sb, in_=wqT.rearrange("(t p) m -> p t m", p=128))
        nc.sync.dma_start(out=wk_sb, in_=wkT.rearrange("(t p) m -> p t m", p=128))
        nc.sync.dma_start(out=wv_sb, in_=wvT.rearrange("(t p) m -> p t m", p=128))
        nc.sync.dma_start(out=wo_sb, in_=woT.rearrange("(t p) m -> p t m", p=128))

        # ---------------- phase 1: projections + RoPE ----------------
        with ExitStack() as p1:
            p1_res = p1.enter_context(tc.tile_pool(name="p1res", bufs=1))
            cos_sb = p1_res.tile([128, S], F32, tag="cos")
            sin_sb = p1_res.tile([128, S], F32, tag="sin")
            nc.sync.dma_start(out=cos_sb, in_=cosT[:, :])
            nc.sync.dma_start(out=sin_sb, in_=sinT[:, :])

            xpool = p1.enter_context(tc.tile_pool(name="xpool", bufs=4))
            rpool = p1.enter_context(tc.tile_pool(name="rope", bufs=3))
            pq_pool = p1.enter_context(tc.tile_pool(name="pq", bufs=2, space="PSUM"))
            pk_pool = p1.enter_context(tc.tile_pool(name="pk", bufs=2, space="PSUM"))
            pvt_pool = p1.enter_context(tc.tile_pool(name="pvt", bufs=1, space="PSUM"))
            pv2_pool = p1.enter_context(tc.tile_pool(name="pv2", bufs=1, space="PSUM"))

            for qc in range(NQ):
                q0 = qc * QC
                pq = pq_pool.tile([128, HPC, QC], F32, tag="pq")
                pk = pk_pool.tile([128, QC], F32, tag="pk")
                pvt = pvt_pool.tile([128, QC], F32, tag="pvt")
                for ki in range(NKIN):
                    xt = xpool.tile([128, QC], F32, tag="xt")
                    nc.sync.dma_start(
                        out=xt, in_=xT[ki * 128:(ki + 1) * 128, q0:q0 + QC])
                    st = ki == 0
                    sp = ki == NKIN - 1
                    for h in range(HPC):
                        nc.tensor.matmul(
                            pq[:, h, :], lhsT=wq_sb[:, ki, h * D:(h + 1) * D],
                            rhs=xt, start=st, stop=sp)
                    nc.tensor.matmul(pk, lhsT=wk_sb[:, ki, :], rhs=xt,
                                     start=st, stop=sp)
                    nc.tensor.matmul(pvt, lhsT=wv_sb[:, ki, :], rhs=xt,
                                     start=st, stop=sp)
                # VT -> V via PE block transposes (each a single full write)
                vtT = rpool.tile([128, QC], F32, tag="vtT")
                nc.vector.tensor_copy(vtT, pvt)
                pv2 = pv2_pool.tile([128, 4, D], F32, tag="pv2")
                for si in range(4):
                    nc.tensor.transpose(
                        pv2[:, si, :], vtT[:, si * 128:(si + 1) * 128], ident)

                # RoPE: dest = src*cos + rot(src)*sin, rot along partitions
                cos_c = cos_sb[:, q0:q0 + QC]
                sin_c = sin_sb[:, q0:q0 + QC]
                for h in range(HPC):
                    src = pq[:, h, :]
                    dst = qt[:, h, q0:q0 + QC]
                    rot = rpool.tile([128, QC], F32, tag="rot")
                    nc.vector.tensor_scalar_mul(rot[0:64, :], src[64:128, :], -1.0)
                    nc.vector.tensor_copy(rot[64:128, :], src[0:64, :])
                    nc.vector.tensor_mul(dst, src, cos_c)
                    nc.vector.tensor_mul(rot, rot, sin_c)
                    nc.vector.tensor_add(dst, dst, rot)
                src = pk
                dst = kt[:, q0:q0 + QC]
                rot = rpool.tile([128, QC], F32, tag="rot")
                nc.vector.tensor_scalar_mul(rot[0:64, :], src[64:128, :], -1.0)
                nc.vector.tensor_copy(rot[64:128, :], src[0:64, :])
                nc.vector.tensor_mul(dst, src, cos_c)
                nc.vector.tensor_mul(rot, rot, sin_c)
                nc.vector.tensor_add(dst, dst, rot)

                nc.vector.tensor_copy(vt[:, qc * 4:(qc + 1) * 4, :], pv2)

        # ---------------- phase 2: attention ----------------
        with ExitStack() as p2:
            wpool = p2.enter_context(tc.tile_pool(name="wpool", bufs=4))
            apool = p2.enter_context(tc.tile_pool(name="acc", bufs=2))
            npool = p2.enter_context(tc.tile_pool(name="norm", bufs=2))
            copool = p2.enter_context(tc.tile_pool(name="cout", bufs=2))
            ps_pool = p2.enter_context(tc.tile_pool(name="ps", bufs=3, space="PSUM"))
            pc_pool = p2.enter_context(tc.tile_pool(name="pc", bufs=2, space="PSUM"))
            pe_pool = p2.enter_context(tc.tile_pool(name="pe", bufs=1, space="PSUM"))
            pb_pool = p2.enter_context(tc.tile_pool(name="pb", bufs=1, space="PSUM"))

            for h in range(HPC):
                for qc in range(NQ):
                    q0 = qc * QC
                    nk = (qc + 1) * 4
                    pc = pc_pool.tile([128, QC], F32, tag="pc")
                    acc = apool.tile([128, QC], F32, tag="acc")
                    for ki in range(nk):
                        k0 = ki * KT
                        ps = ps_pool.tile([128, QC], F32, tag="ps")
                        nc.tensor.matmul(ps, lhsT=kt[:, k0:k0 + KT],
                                         rhs=qt[:, h, q0:q0 + QC],
                                         start=True, stop=True)
                        wt = wpool.tile([128, QC], F32, tag="wt")
                        nc.scalar.activation(wt, ps,
                                             mybir.ActivationFunctionType.Exp,
                                             scale=INV_SQRT_D)
                        if k0 + KT - 1 > q0:
                            # keep where (q0+j) - (k0+p) >= 0
                            nc.gpsimd.affine_select(
                                out=wt, in_=wt, pattern=[[1, QC]],
                                compare_op=mybir.AluOpType.is_ge, fill=0.0,
                                base=q0 - k0, channel_multiplier=-1)
                        nc.tensor.matmul(pc, lhsT=vt[:, ki, :], rhs=wt,
                                         start=(ki == 0), stop=(ki == nk - 1))
                        if ki == 0:
                            nc.vector.tensor_copy(acc, wt)
                        else:
                            nc.vector.tensor_add(acc, acc, wt)
                    pe = pe_pool.tile([1, QC], F32, tag="pe")
                    nc.tensor.matmul(pe, lhsT=ones_k, rhs=acc,
                                     start=True, stop=True)
                    rec = npool.tile([1, QC], F32, tag="rec")
                    nc.vector.reciprocal(rec, pe)
                    pb = pb_pool.tile([128, QC], F32, tag="pb")
                    nc.tensor.matmul(pb, lhsT=ones_r, rhs=rec,
                                     start=True, stop=True)
                    bc = npool.tile([128, QC], F32, tag="bc")
                    nc.vector.tensor_copy(bc, pb)
                    cout = copool.tile([128, QC], F32, tag="cout")
                    nc.vector.tensor_mul(cout, pc, bc)
                    nc.sync.dma_start(
                        out=ctx_local[h * D:(h + 1) * D, q0:q0 + QC], in_=cout)

        if debug:
            nc.sync.dma_start(out=dbg_qt[:, :], in_=qt[:, 0, :])
            nc.sync.dma_start(out=dbg_kt[:, :], in_=kt)
            nc.sync.dma_start(out=dbg_vt[:, :, :], in_=vt)

        # ---------------- allgather ----------------
        tc.strict_bb_all_engine_barrier()
        nc.gpsimd.collective_compute(
            "AllGather",
            mybir.AluOpType.bypass,
            replica_groups=[list(range(NCORES))],
            ins=[ctx_local[:, :]],
            outs=[ctx_full[:, :]],
        )
        tc.strict_bb_all_engine_barrier()
        if debug:
            nc.sync.dma_start(out=dbg_cl[:, :], in_=ctx_local[:, :])
            nc.sync.dma_start(out=dbg_cf[:, :], in_=ctx_full[:, :])

        # ---------------- phase 3: out-proj ----------------
        with ExitStack() as p3:
            cpool = p3.enter_context(tc.tile_pool(name="cpool", bufs=4))
            opool = p3.enter_context(tc.tile_pool(name="opool", bufs=2))
            po_pool = p3.enter_context(tc.tile_pool(name="po", bufs=2, space="PSUM"))
            for sc in range(NQ):
                s0 = sc * QC
                po = po_pool.tile([128, HPC, QC], F32, tag="po")
                for ti in range(NKIN):
                    ct = cpool.tile([128, QC], F32, tag="ct")
                    nc.sync.dma_start(
                        out=ct, in_=ctx_full[ti * 128:(ti + 1) * 128, s0:s0 + QC])
                    for m in range(HPC):
                        nc.tensor.matmul(
                            po[:, m, :], lhsT=wo_sb[:, ti, m * D:(m + 1) * D],
                            rhs=ct, start=(ti == 0), stop=(ti == NKIN - 1))
                ot = opool.tile([128, HPC, QC], F32, tag="ot")
                nc.vector.tensor_copy(ot, po)
                for m in range(HPC):
                    nc.sync.dma_start(
                        out=outT[m * 128:(m + 1) * 128, s0:s0 + QC],
                        in_=ot[:, m, :])

    _split_sync_waits(nc)
    return nc


_NC_CACHE = None


def _get_nc():
    global _NC_CACHE
    if _NC_CACHE is None:
        _NC_CACHE = build_nc()
    return _NC_CACHE


def _make_in_maps(x, cos, sin, Wq, Wk, Wv, Wo):
    xT = np.ascontiguousarray(x.reshape(S, DIN).T)
    cosT = np.ascontiguousarray(cos.T)
    sinT = np.ascontiguousarray(sin.T)
    in_maps = []
    for c in range(NCORES):
        g = c // 2
        in_maps.append({
            "xT": xT,
            "wqT": np.ascontiguousarray(Wq[c * 256:(c + 1) * 256, :].T),
            "wkT": np.ascontiguousarray(Wk[g * 128:(g + 1) * 128, :].T),
            "wvT": np.ascontiguousarray(Wv[g * 128:(g + 1) * 128, :].T),
            "cosT": cosT,
            "sinT": sinT,
            "woT": np.ascontiguousarray(Wo[c * 256:(c + 1) * 256, :].T),
        })
    return in_maps


def run(x, cos, sin, Wq, Wk, Wv, Wo, trace=False):
    nc = _get_nc()
    in_maps = _make_in_maps(x, cos, sin, Wq, Wk, Wv, Wo)
    res = run_bass_kernel_spmd(nc, in_maps, list(range(NCORES)), trace=trace)
    outT = np.concatenate([res.results[c]["outT"] for c in range(NCORES)], axis=0)
    out = np.ascontiguousarray(outT.T).reshape(1, S, DIN).astype(np.float32)
    return out, res


def kernel(x, mask, cos, sin, Wq, Wk, Wv, Wo):
    out, _ = run(np.asarray(x, dtype=np.float32), np.asarray(cos, np.float32),
                 np.asarray(sin, np.float32), np.asarray(Wq, np.float32),
                 np.asarray(Wk, np.float32), np.asarray(Wv, np.float32),
                 np.asarray(Wo, np.float32))
    return out



# revision 8
# speedup vs baseline: 2.4077x; 2.4077x over previous
"""GQA kernel for Trainium2, 8 NeuronCores.

Problem: x[1,4096,2048], H=16 heads, G=4 kv-groups, D=128, causal mask,
RoPE on q/k, out-proj. Sharding: 2 heads per core (core c -> heads 2c,2c+1,
kv-group c//2). All tensors fed pre-transposed so matmul contractions land
on SBUF partitions. fp16 on-chip (PSUM accumulation fp32); exp is computed
as exp(s/sqrt(D) - 6*ln2) so weights fit fp16; the 2^-6 scale cancels in
the softmax normalization.

Structure per core:
  proj:  QT/KT/V projections from xT (streamed once) + RoPE, all chunks.
  attn:  per-head causal attention in scoresT [k,q] orientation, q-chunks
         in DESCENDING order; per-chunk ctx written to a DRAM tile and
         AllGathered (8 small collectives, dependency-tracked, overlapped
         with later attention); softmax denom accumulated on DVE+GpSimd,
         reduced by ones-matmul, inverted by reciprocal_approx_fast,
         broadcast by PE matmul.
  outp:  column-parallel out-proj per gathered chunk, interleaved into the
         attention loop one chunk behind its collective.

Output per core: outT_c = out.T[c*256:(c+1)*256, :]; host concatenates and
transposes back.
"""

import sys

for _p in ("/opt/trn_rl_repo",):
    if _p not in sys.path:
        sys.path.append(_p)

from contextlib import ExitStack

import numpy as np

import concourse.bass as bass
import concourse.tile as tile
from concourse import masks, mybir
from concourse.bass_utils import run_bass_kernel_spmd

F32 = mybir.dt.float32
F32R = mybir.dt.float32r
F16 = mybir.dt.float16
S = 4096
MAX_WAITS = 1  # walrus CoreV3 rejects instructions with more sync waits


def _split_sync_waits(nc, maxw=MAX_WAITS):
    """Hoist excess sem waits onto NOPs inserted before the instruction on
    the same engine queue (queue order makes this equivalent)."""
    from concourse import mybir as mb
    n = 0
    for bassbb in nc.bb_map.values():
        bb = bassbb.bb
        insts = list(bb.instructions)
        out = []
        changed = False
        for ins in insts:
            si = ins.sync_info
            if si is not None and si.on_wait and len(si.on_wait) > maxw:
                waits = list(si.on_wait)
                head, rest = waits[:-maxw], waits[-maxw:]
                while head:
                    chunk, head = head[:maxw], head[maxw:]
                    n += 1
                    nop = mb.InstNoOp(
                        name=f"I-ws{n}",
                        engine=ins.engine,
                        ins=[],
                        outs=[],
                        sync_info=mb.SyncInfo(on_wait=chunk, on_update=[]),
                    )
                    nc.register_instruction(nop)
                    out.append(nop)
                ins.sync_info = mb.SyncInfo(
                    on_wait=rest, on_update=list(si.on_update or []))
                changed = True
            out.append(ins)
        if changed:
            try:
                bb.instructions[:] = out
            except TypeError:
                bb.set_instructions(out)
    return n


DIN = 2048
D = 128
HPC = 2          # heads per core
NCORES = 8
QC = 512         # q-chunk (free dim per matmul)
NQ = S // QC     # 8 q-chunks
KT = 128         # k tile (partition dim)
NKIN = DIN // 128  # 16 contraction tiles for projections
INV_SQRT_D = 1.0 / np.sqrt(D)
EXP_BIAS = float(-6.0 * np.log(2.0))  # 2^-6 scale on exp; cancels in softmax


def build_nc():
    nc = bass.Bass(num_devices=NCORES)

    xT = nc.dram_tensor("xT", [DIN, S], F16, kind="ExternalInput")
    wqT = nc.dram_tensor("wqT", [DIN, HPC * D], F16, kind="ExternalInput")
    wkT = nc.dram_tensor("wkT", [DIN, D], F16, kind="ExternalInput")
    wvT = nc.dram_tensor("wvT", [DIN, D], F16, kind="ExternalInput")
    cosT = nc.dram_tensor("cosT", [D, S], F16, kind="ExternalInput")
    sinT = nc.dram_tensor("sinT", [D, S], F16, kind="ExternalInput")
    woT = nc.dram_tensor("woT", [DIN, HPC * D], F16, kind="ExternalInput")
    outT = nc.dram_tensor("outT", [HPC * D, S], F32, kind="ExternalOutput")

    with ExitStack() as ctx:
        tc = ctx.enter_context(tile.TileContext(nc))

        res = ctx.enter_context(tc.tile_pool(name="res", bufs=1))
        dram = ctx.enter_context(tc.tile_pool(name="dram", bufs=1, space="DRAM"))

        # collective bounce tiles (dependency-tracked DRAM tiles)
        ctx_loc = [dram.tile([HPC * D, QC], F16, tag=f"cl{qc}", name=f"cl{qc}")
                   for qc in range(NQ)]
        ctx_ful = [dram.tile([NCORES * HPC * D, QC], F16, tag=f"cf{qc}",
                             name=f"cf{qc}")
                   for qc in range(NQ)]

        # resident SBUF tensors
        qt = res.tile([128, HPC, S], F16, tag="qt")          # QT per head
        kt = res.tile([128, S], F16, tag="kt")               # KT (shared group)
        vt = res.tile([128, S // 128, D], F16, tag="vt")     # V as s-tiles
        wq_sb = res.tile([128, NKIN, HPC * D], F16, tag="wq")
        wk_sb = res.tile([128, NKIN, D], F16, tag="wk")
        wv_sb = res.tile([128, NKIN, D], F16, tag="wv")
        wo_sb = res.tile([128, NKIN, HPC * D], F16, tag="wo")
        cos_sb = res.tile([128, S], F16, tag="cos")
        sin_sb = res.tile([128, S], F16, tag="sin")
        ones_k = res.tile([128, 1], F16, tag="ones_k")       # lhsT for col sums
        ebias = res.tile([128, 1], F32, tag="ebias")         # exp bias 2^-6
        ones_r = res.tile([1, 128], F16, tag="ones_r")       # lhsT for bcast
        ident = res.tile([128, 128], F16, tag="ident")       # PE transpose id

        nc.vector.memset(ones_k, 1.0)
        nc.vector.memset(ebias, EXP_BIAS)
        nc.vector.memset(ones_r, 1.0)
        masks.make_identity(nc, ident)

        # weight loads
        nc.sync.dma_start(out=wq_sb, in_=wqT.rearrange("(t p) m -> p t m", p=128))
        nc.sync.dma_start(out=wk_sb, in_=wkT.rearrange("(t p) m -> p t m", p=128))
        nc.sync.dma_start(out=wv_sb, in_=wvT.rearrange("(t p) m -> p t m", p=128))
        nc.sync.dma_start(out=wo_sb, in_=woT.rearrange("(t p) m -> p t m", p=128))
        nc.sync.dma_start(out=cos_sb, in_=cosT[:, :])
        nc.sync.dma_start(out=sin_sb, in_=sinT[:, :])

        def rope(dst, src, rpool, cos_c, sin_c):
            # dst = src*cos + rot(src)*sin, rotate-half along partitions
            rot = rpool.tile([128, QC], F16, tag="rot")
            nc.vector.tensor_scalar_mul(rot[0:64, :], src[64:128, :], -1.0)
            nc.vector.tensor_copy(rot[64:128, :], src[0:64, :])
            nc.vector.tensor_mul(dst, src, cos_c)
            nc.vector.tensor_mul(rot, rot, sin_c)
            nc.vector.tensor_add(dst, dst, rot)

        # ---------------- projections + RoPE ----------------
        with ExitStack() as p1:
            xpool = p1.enter_context(tc.tile_pool(name="xpool", bufs=4))
            rpool = p1.enter_context(tc.tile_pool(name="rope", bufs=3))
            pp_pool = p1.enter_context(tc.tile_pool(name="pp", bufs=2, space="PSUM"))
            pv2_pool = p1.enter_context(tc.tile_pool(name="pv2", bufs=1, space="PSUM"))

            for qc in range(NQ):
                q0 = qc * QC
                pq = pp_pool.tile([128, 2, QC], F32, tag="pp")
                pkv = pp_pool.tile([128, 2, QC], F32, tag="pp")
                for ki in range(NKIN):
                    xt = xpool.tile([128, QC], F16, tag="xt")
                    nc.sync.dma_start(
                        out=xt, in_=xT[ki * 128:(ki + 1) * 128, q0:q0 + QC])
                    st = ki == 0
                    sp = ki == NKIN - 1
                    for h in range(HPC):
                        nc.tensor.matmul(
                            pq[:, h, :], lhsT=wq_sb[:, ki, h * D:(h + 1) * D],
                            rhs=xt, start=st, stop=sp)
                    nc.tensor.matmul(pkv[:, 0, :], lhsT=wk_sb[:, ki, :], rhs=xt,
                                     start=st, stop=sp)
                    nc.tensor.matmul(pkv[:, 1, :], lhsT=wv_sb[:, ki, :], rhs=xt,
                                     start=st, stop=sp)
                # VT -> V via PE block transposes
                vtT = rpool.tile([128, QC], F16, tag="vtT")
                nc.vector.tensor_copy(vtT, pkv[:, 1, :])
                pv2 = pv2_pool.tile([128, 4, D], F16, tag="pv2")
                for si in range(4):
                    nc.tensor.transpose(
                        pv2[:, si, :], vtT[:, si * 128:(si + 1) * 128], ident)
                nc.vector.tensor_copy(vt[:, qc * 4:(qc + 1) * 4, :], pv2)

                cos_c = cos_sb[:, q0:q0 + QC]
                sin_c = sin_sb[:, q0:q0 + QC]
                rope(kt[:, q0:q0 + QC], pkv[:, 0, :], rpool, cos_c, sin_c)
                for h in range(HPC):
                    rope(qt[:, h, q0:q0 + QC], pq[:, h, :], rpool, cos_c, sin_c)

        # ---------------- attention (+ interleaved out-proj) ----------------
        with ExitStack() as p2:
            wpool = p2.enter_context(tc.tile_pool(name="wpool", bufs=4))
            apool = p2.enter_context(tc.tile_pool(name="acc", bufs=2))
            npool = p2.enter_context(tc.tile_pool(name="norm", bufs=2))
            copool = p2.enter_context(tc.tile_pool(name="cout", bufs=2))
            cpool = p2.enter_context(tc.tile_pool(name="cpool", bufs=4))
            opool = p2.enter_context(tc.tile_pool(name="opool", bufs=2))
            ps_pool = p2.enter_context(tc.tile_pool(name="ps", bufs=2, space="PSUM"))
            pc_pool = p2.enter_context(tc.tile_pool(name="pc", bufs=1, space="PSUM"))
            aux_pool = p2.enter_context(tc.tile_pool(name="aux", bufs=1, space="PSUM"))
            po_pool = p2.enter_context(tc.tile_pool(name="po", bufs=1, space="PSUM"))

            def attn_chunk(qc):
                q0 = qc * QC
                nk = (qc + 1) * 4
                for h in range(HPC):
                    pc = pc_pool.tile([128, QC], F32, tag="pc")
                    acc = apool.tile([128, QC], F16, tag="acc")
                    acc2 = apool.tile([128, QC], F16, tag="acc2")
                    for g in range(nk // 2):
                        ps = ps_pool.tile([128, 2, QC], F32, tag="ps")
                        wt = wpool.tile([128, 2, QC], F16, tag="wt")
                        for j in range(2):
                            k0 = (2 * g + j) * KT
                            nc.tensor.matmul(ps[:, j, :], lhsT=kt[:, k0:k0 + KT],
                                             rhs=qt[:, h, q0:q0 + QC],
                                             start=True, stop=True)
                        nc.scalar.activation(wt, ps,
                                             mybir.ActivationFunctionType.Exp,
                                             scale=INV_SQRT_D, bias=ebias[:, :])
                        for j in range(2):
                            ki = 2 * g + j
                            k0 = ki * KT
                            if k0 + KT - 1 > q0:
                                # keep where (q0+col) - (k0+p) >= 0
                                nc.gpsimd.affine_select(
                                    out=wt[:, j, :], in_=wt[:, j, :],
                                    pattern=[[1, QC]],
                                    compare_op=mybir.AluOpType.is_ge, fill=0.0,
                                    base=q0 - k0, channel_multiplier=-1)
                            nc.tensor.matmul(pc, lhsT=vt[:, ki, :],
                                             rhs=wt[:, j, :],
                                             start=(ki == 0), stop=(ki == nk - 1))
                        # softmax denominator partials, split across engines
                        if g == 0:
                            nc.vector.tensor_copy(acc, wt[:, 0, :])
                            nc.gpsimd.tensor_copy(acc2, wt[:, 1, :])
                        else:
                            nc.vector.tensor_add(acc, acc, wt[:, 0, :])
                            nc.gpsimd.tensor_add(acc2, acc2, wt[:, 1, :])
                    aux = aux_pool.tile([128, QC], F32, tag="aux")
                    nc.tensor.matmul(aux[0:1, :], lhsT=ones_k, rhs=acc,
                                     start=True, stop=False)
                    nc.tensor.matmul(aux[0:1, :], lhsT=ones_k, rhs=acc2,
                                     start=False, stop=True)
                    # 1/denom via exp(-ln(denom)) on the ACT engine
                    lden = npool.tile([1, QC], F32, tag="lden")
                    nc.scalar.activation(lden, aux[0:1, :],
                                         mybir.ActivationFunctionType.Ln)
                    rec16 = npool.tile([1, QC], F16, tag="rec16")
                    nc.scalar.activation(rec16, lden,
                                         mybir.ActivationFunctionType.Exp,
                                         scale=-1.0)
                    nc.tensor.matmul(aux[:, :], lhsT=ones_r, rhs=rec16,
                                     start=True, stop=True)
                    bc = npool.tile([128, QC], F32, tag="bc")
                    nc.vector.tensor_copy(bc, aux)
                    cout = copool.tile([128, QC], F16, tag="cout")
                    nc.vector.tensor_mul(cout, pc, bc)
                    nc.sync.dma_start(
                        out=ctx_loc[qc][h * D:(h + 1) * D, :], in_=cout)
                nc.gpsimd.collective_compute(
                    "AllGather",
                    mybir.AluOpType.bypass,
                    replica_groups=[list(range(NCORES))],
                    ins=[ctx_loc[qc].opt()],
                    outs=[ctx_ful[qc].opt()],
                )

            def outp_chunk(qc):
                po = po_pool.tile([128, HPC, QC], F32, tag="po")
                for ti in range(NKIN):
                    ct = cpool.tile([128, QC], F16, tag="ct")
                    nc.sync.dma_start(
                        out=ct, in_=ctx_ful[qc][ti * 128:(ti + 1) * 128, :])
                    for m in range(HPC):
                        nc.tensor.matmul(
                            po[:, m, :], lhsT=wo_sb[:, ti, m * D:(m + 1) * D],
                            rhs=ct, start=(ti == 0), stop=(ti == NKIN - 1))
                ot = opool.tile([128, HPC, QC], F32, tag="ot")
                nc.vector.tensor_copy(ot, po)
                for m in range(HPC):
                    nc.sync.dma_start(
                        out=outT[m * 128:(m + 1) * 128, qc * QC:(qc + 1) * QC],
                        in_=ot[:, m, :])

            # big chunks first: their collectives complete while smaller
            # chunks compute; out-proj trails one chunk behind.
            for qc in range(NQ - 1, -1, -1):
                attn_chunk(qc)
                if qc + 1 < NQ:
                    outp_chunk(qc + 1)
            outp_chunk(0)

    _split_sync_waits(nc)
    return nc


_NC_CACHE = None


def _get_nc():
    global _NC_CACHE
    if _NC_CACHE is None:
        _NC_CACHE = build_nc()
    return _NC_CACHE


def _make_in_maps(x, cos, sin, Wq, Wk, Wv, Wo):
    f16 = np.float16
    xT = np.ascontiguousarray(x.reshape(S, DIN).T.astype(f16))
    cosT = np.ascontiguousarray(cos.T.astype(f16))
    sinT = np.ascontiguousarray(sin.T.astype(f16))
    in_maps = []
    for c in range(NCORES):
        g = c // 2
        in_maps.append({
            "xT": xT,
            "wqT": np.ascontiguousarray(Wq[c * 256:(c + 1) * 256, :].T.astype(f16)),
            "wkT": np.ascontiguousarray(Wk[g * 128:(g + 1) * 128, :].T.astype(f16)),
            "wvT": np.ascontiguousarray(Wv[g * 128:(g + 1) * 128, :].T.astype(f16)),
            "cosT": cosT,
            "sinT": sinT,
            "woT": np.ascontiguousarray(Wo[c * 256:(c + 1) * 256, :].T.astype(f16)),
        })
    return in_maps


def run(x, cos, sin, Wq, Wk, Wv, Wo, trace=False):
    nc = _get_nc()
    in_maps = _make_in_maps(x, cos, sin, Wq, Wk, Wv, Wo)
    res = run_bass_kernel_spmd(nc, in_maps, list(range(NCORES)), trace=trace)
    outT = np.concatenate([res.results[c]["outT"] for c in range(NCORES)], axis=0)
    out = np.ascontiguousarray(outT.T).reshape(1, S, DIN).astype(np.float32)
    return out, res


def kernel(x, mask, cos, sin, Wq, Wk, Wv, Wo):
    out, _ = run(np.asarray(x, dtype=np.float32), np.asarray(cos, np.float32),
                 np.asarray(sin, np.float32), np.asarray(Wq, np.float32),
                 np.asarray(Wk, np.float32), np.asarray(Wv, np.float32),
                 np.asarray(Wo, np.float32))
    return out


# revision 12
# speedup vs baseline: 2.5420x; 1.0558x over previous
"""GQA kernel for Trainium2, 8 NeuronCores.

Problem: x[1,4096,2048], H=16 heads, G=4 kv-groups, D=128, causal mask,
RoPE on q/k, out-proj. Sharding: 2 heads per core (core c -> heads 2c,2c+1,
kv-group c//2). All tensors fed pre-transposed so matmul contractions land
on SBUF partitions. fp16 on-chip (PSUM accumulation fp32); exp is computed
as exp(s/sqrt(D) - 6*ln2) so weights fit fp16; the 2^-6 scale cancels in
the softmax normalization.

Structure per core:
  proj:  QT/KT/V projections from xT (streamed once) + RoPE, all chunks.
  attn:  per-head causal attention in scoresT [k,q] orientation, q-chunks
         in DESCENDING order; per-chunk ctx written to a DRAM tile and
         AllGathered (8 small collectives, dependency-tracked, overlapped
         with later attention); softmax denom accumulated on DVE+GpSimd,
         reduced by ones-matmul, inverted by reciprocal_approx_fast,
         broadcast by PE matmul.
  outp:  column-parallel out-proj per gathered chunk, interleaved into the
         attention loop one chunk behind its collective.

Output per core: outT_c = out.T[c*256:(c+1)*256, :]; host concatenates and
transposes back.
"""

import sys

for _p in ("/opt/trn_rl_repo",):
    if _p not in sys.path:
        sys.path.append(_p)

from contextlib import ExitStack

import numpy as np

import concourse.bass as bass
import concourse.tile as tile
from concourse import masks, mybir
from concourse.bass_utils import run_bass_kernel_spmd

F32 = mybir.dt.float32
F32R = mybir.dt.float32r
F16 = mybir.dt.float16
S = 4096
MAX_WAITS = 1  # walrus CoreV3 rejects instructions with more sync waits


def _split_sync_waits(nc, maxw=MAX_WAITS):
    """Hoist excess sem waits onto NOPs inserted before the instruction on
    the same engine queue (queue order makes this equivalent)."""
    from concourse import mybir as mb
    n = 0
    for bassbb in nc.bb_map.values():
        bb = bassbb.bb
        insts = list(bb.instructions)
        out = []
        changed = False
        for ins in insts:
            si = ins.sync_info
            if si is not None and si.on_wait and len(si.on_wait) > maxw:
                waits = list(si.on_wait)
                head, rest = waits[:-maxw], waits[-maxw:]
                while head:
                    chunk, head = head[:maxw], head[maxw:]
                    n += 1
                    nop = mb.InstNoOp(
                        name=f"I-ws{n}",
                        engine=ins.engine,
                        ins=[],
                        outs=[],
                        sync_info=mb.SyncInfo(on_wait=chunk, on_update=[]),
                    )
                    nc.register_instruction(nop)
                    out.append(nop)
                ins.sync_info = mb.SyncInfo(
                    on_wait=rest, on_update=list(si.on_update or []))
                changed = True
            out.append(ins)
        if changed:
            try:
                bb.instructions[:] = out
            except TypeError:
                bb.set_instructions(out)
    return n


DIN = 2048
D = 128
HPC = 2          # heads per core
NCORES = 8
QC = 512         # q-chunk (free dim per matmul)
NQ = S // QC     # 8 q-chunks
KT = 128         # k tile (partition dim)
NKIN = DIN // 128  # 16 contraction tiles for projections
INV_SQRT_D = 1.0 / np.sqrt(D)
EXP_BIAS = float(-6.0 * np.log(2.0))  # 2^-6 scale on exp; cancels in softmax


def build_nc():
    nc = bass.Bass(num_devices=NCORES)

    xT = nc.dram_tensor("xT", [DIN, S], F16, kind="ExternalInput")
    wqT = nc.dram_tensor("wqT", [DIN, HPC * D], F16, kind="ExternalInput")
    wkT = nc.dram_tensor("wkT", [DIN, D], F16, kind="ExternalInput")
    wvT = nc.dram_tensor("wvT", [DIN, D], F16, kind="ExternalInput")
    cosT = nc.dram_tensor("cosT", [D, S], F16, kind="ExternalInput")
    sinT = nc.dram_tensor("sinT", [D, S], F16, kind="ExternalInput")
    woT = nc.dram_tensor("woT", [DIN, HPC * D], F16, kind="ExternalInput")
    outT = nc.dram_tensor("outT", [HPC * D, S], F32, kind="ExternalOutput")

    with ExitStack() as ctx:
        tc = ctx.enter_context(tile.TileContext(nc))

        res = ctx.enter_context(tc.tile_pool(name="res", bufs=1))
        dram = ctx.enter_context(tc.tile_pool(name="dram", bufs=1, space="DRAM"))

        # collective bounce tiles (dependency-tracked DRAM tiles)
        ctx_loc = [dram.tile([HPC * D, QC], F16, tag=f"cl{qc}", name=f"cl{qc}")
                   for qc in range(NQ)]
        ctx_ful = [dram.tile([NCORES * HPC * D, QC], F16, tag=f"cf{qc}",
                             name=f"cf{qc}")
                   for qc in range(NQ)]

        # resident SBUF tensors
        qt = res.tile([128, HPC, S], F16, tag="qt")          # QT per head
        kt = res.tile([128, S], F16, tag="kt")               # KT (shared group)
        vt = res.tile([128, S // 128, D], F16, tag="vt")     # V as s-tiles
        wq_sb = res.tile([128, NKIN, HPC * D], F16, tag="wq")
        wk_sb = res.tile([128, NKIN, D], F16, tag="wk")
        wv_sb = res.tile([128, NKIN, D], F16, tag="wv")
        wo_sb = res.tile([128, NKIN, HPC * D], F16, tag="wo")
        cos_sb = res.tile([128, S], F16, tag="cos")
        sin_sb = res.tile([128, S], F16, tag="sin")
        ones_k = res.tile([128, 1], F16, tag="ones_k")       # lhsT for col sums
        ebias = res.tile([128, 1], F32, tag="ebias")         # exp bias 2^-6
        ones_r = res.tile([1, 128], F16, tag="ones_r")       # lhsT for bcast
        ident = res.tile([128, 128], F16, tag="ident")       # PE transpose id

        nc.vector.memset(ones_k, 1.0)
        nc.vector.memset(ebias, EXP_BIAS)
        nc.vector.memset(ones_r, 1.0)
        masks.make_identity(nc, ident)

        # weight loads
        nc.sync.dma_start(out=wq_sb, in_=wqT.rearrange("(t p) m -> p t m", p=128))
        nc.sync.dma_start(out=wk_sb, in_=wkT.rearrange("(t p) m -> p t m", p=128))
        nc.sync.dma_start(out=wv_sb, in_=wvT.rearrange("(t p) m -> p t m", p=128))
        nc.sync.dma_start(out=wo_sb, in_=woT.rearrange("(t p) m -> p t m", p=128))
        nc.sync.dma_start(out=cos_sb, in_=cosT[:, :])
        nc.sync.dma_start(out=sin_sb, in_=sinT[:, :])

        def rope(dst, src, rpool, cos_c, sin_c):
            # dst = src*cos + rot(src)*sin, rotate-half along partitions
            rot = rpool.tile([128, QC], F16, tag="rot")
            nc.vector.tensor_scalar_mul(rot[0:64, :], src[64:128, :], -1.0)
            nc.vector.tensor_copy(rot[64:128, :], src[0:64, :])
            nc.vector.tensor_mul(dst, src, cos_c)
            nc.vector.tensor_mul(rot, rot, sin_c)
            nc.vector.tensor_add(dst, dst, rot)

        # ---------------- projections + RoPE ----------------
        with ExitStack() as p1:
            xpool = p1.enter_context(tc.tile_pool(name="xpool", bufs=4))
            rpool = p1.enter_context(tc.tile_pool(name="rope", bufs=3))
            pp_pool = p1.enter_context(tc.tile_pool(name="pp", bufs=3, space="PSUM"))
            pv2_pool = p1.enter_context(tc.tile_pool(name="pv2", bufs=1, space="PSUM"))

            def transpose_v(qc, vtT):
                # VT -> V via PE block transposes (lagged one chunk so the PE
                # never waits on the vtT drain copy)
                pv2 = pv2_pool.tile([128, 4, D], F16, tag="pv2")
                for si in range(4):
                    nc.tensor.transpose(
                        pv2[:, si, :], vtT[:, si * 128:(si + 1) * 128], ident)
                nc.vector.tensor_copy(vt[:, qc * 4:(qc + 1) * 4, :], pv2)

            pend_tv = None
            for qc in range(NQ):
                q0 = qc * QC
                pq = pp_pool.tile([128, 2, QC], F32, tag="pp")
                pkv = pp_pool.tile([128, 2, QC], F32, tag="pp")
                for ki in range(NKIN):
                    xt = xpool.tile([128, QC], F16, tag="xt")
                    nc.sync.dma_start(
                        out=xt, in_=xT[ki * 128:(ki + 1) * 128, q0:q0 + QC])
                    st = ki == 0
                    sp = ki == NKIN - 1
                    for h in range(HPC):
                        nc.tensor.matmul(
                            pq[:, h, :], lhsT=wq_sb[:, ki, h * D:(h + 1) * D],
                            rhs=xt, start=st, stop=sp)
                    nc.tensor.matmul(pkv[:, 0, :], lhsT=wk_sb[:, ki, :], rhs=xt,
                                     start=st, stop=sp)
                    nc.tensor.matmul(pkv[:, 1, :], lhsT=wv_sb[:, ki, :], rhs=xt,
                                     start=st, stop=sp)
                    if ki == 2 and pend_tv is not None:
                        transpose_v(*pend_tv)
                        pend_tv = None
                vtT = rpool.tile([128, QC], F16, tag="vtT")
                nc.vector.tensor_copy(vtT, pkv[:, 1, :])
                pend_tv = (qc, vtT)

                cos_c = cos_sb[:, q0:q0 + QC]
                sin_c = sin_sb[:, q0:q0 + QC]
                rope(kt[:, q0:q0 + QC], pkv[:, 0, :], rpool, cos_c, sin_c)
                for h in range(HPC):
                    rope(qt[:, h, q0:q0 + QC], pq[:, h, :], rpool, cos_c, sin_c)
            transpose_v(*pend_tv)

        # ---------------- attention (+ interleaved out-proj) ----------------
        with ExitStack() as p2:
            wpool = p2.enter_context(tc.tile_pool(name="wpool", bufs=4))
            npool = p2.enter_context(tc.tile_pool(name="norm", bufs=2))
            copool = p2.enter_context(tc.tile_pool(name="cout", bufs=2))
            cpool = p2.enter_context(tc.tile_pool(name="cpool", bufs=2))
            opool = p2.enter_context(tc.tile_pool(name="opool", bufs=2))
            ps_pool = p2.enter_context(tc.tile_pool(name="ps", bufs=2, space="PSUM"))
            pc_pool = p2.enter_context(tc.tile_pool(name="pc", bufs=2, space="PSUM"))
            aux_pool = p2.enter_context(tc.tile_pool(name="aux", bufs=1, space="PSUM"))
            # one shared bank ring: out-proj accumulators and the softmax
            # broadcast both use it transiently
            pb_pool = p2.enter_context(tc.tile_pool(name="pb", bufs=1, space="PSUM"))

            # Pending normalize work, lagged one head so its serial
            # ACT chain hides behind the next head's matmuls.
            pend_norm = [None, None]  # [free_aux_fn, rest_fn]

            def attn_chunk(qc):
                q0 = qc * QC
                nk = (qc + 1) * 4
                ng = nk // 2
                for h in range(HPC):
                    pc = pc_pool.tile([128, QC], F32, tag="pc")
                    aux = None
                    prev = None  # (wt, g) waiting for its pc/denominator mms

                    def consume(wt, g):
                        # pc + denominator matmuls for group g (lagged one
                        # group behind the score matmuls)
                        nonlocal aux
                        if aux is None:
                            aux = aux_pool.tile([128, QC], F32, tag="aux")
                        for j in range(2):
                            ki = 2 * g + j
                            nc.tensor.matmul(pc, lhsT=vt[:, ki, :],
                                             rhs=wt[:, j, :],
                                             start=(ki == 0), stop=(ki == nk - 1))
                            # column sums: j=0 -> partition 0, j=1 -> 32
                            nc.tensor.matmul(
                                aux[32 * j:32 * j + 1, :], lhsT=ones_k,
                                rhs=wt[:, j, :],
                                start=(g == 0), stop=(g == ng - 1))

                    for g in range(ng):
                        ps = ps_pool.tile([128, 2, QC], F32, tag="ps")
                        wt = wpool.tile([128, 2, QC], F16, tag="wt")
                        for j in range(2):
                            k0 = (2 * g + j) * KT
                            nc.tensor.matmul(ps[:, j, :], lhsT=kt[:, k0:k0 + KT],
                                             rhs=qt[:, h, q0:q0 + QC],
                                             start=True, stop=True)
                        if g == 0 and pend_norm[0] is not None:
                            pend_norm[0]()  # drain prev head's aux bank
                            pend_norm[0] = None
                        if prev is not None:
                            consume(*prev)
                        if g == 1 and pend_norm[1] is not None:
                            pend_norm[1]()  # rest of prev head's normalize
                            pend_norm[1] = None
                        nc.scalar.activation(wt, ps,
                                             mybir.ActivationFunctionType.Exp,
                                             scale=INV_SQRT_D, bias=ebias[:, :])
                        for j in range(2):
                            k0 = (2 * g + j) * KT
                            if k0 + KT - 1 > q0:
                                # keep where (q0+col) - (k0+p) >= 0
                                nc.gpsimd.affine_select(
                                    out=wt[:, j, :], in_=wt[:, j, :],
                                    pattern=[[1, QC]],
                                    compare_op=mybir.AluOpType.is_ge, fill=0.0,
                                    base=q0 - k0, channel_multiplier=-1)
                        prev = (wt, g)
                    consume(*prev)
                    for fn in pend_norm:
                        if fn is not None:
                            fn()
                    pend_norm[0] = make_norm_a(qc, h, aux)
                    pend_norm[1] = make_norm_b(qc, h, pc)

            norm_state = {}

            def make_norm_a(qc, h, aux):
                def fire():
                    # denominator = j0 sums + j1 sums; frees the aux bank
                    den32 = npool.tile([1, QC], F32, tag="den32", name="den32")
                    nc.vector.tensor_copy(den32, aux[0:1, :])
                    nc.vector.tensor_add(den32, den32, aux[32:33, :])
                    norm_state[(qc, h)] = den32
                return fire

            def make_norm_b(qc, h, pc):
                def fire():
                    den32 = norm_state.pop((qc, h))
                    # 1/denom via exp(-ln(denom)) on the ACT engine
                    lden = npool.tile([1, QC], F32, tag="lden", name="lden")
                    nc.scalar.activation(lden, den32,
                                         mybir.ActivationFunctionType.Ln)
                    rec16 = npool.tile([1, QC], F16, tag="rec16", name="rec16")
                    nc.scalar.activation(rec16, lden,
                                         mybir.ActivationFunctionType.Exp,
                                         scale=-1.0)
                    pb = pb_pool.tile([128, QC], F32, tag="pb", name="pb")
                    nc.tensor.matmul(pb, lhsT=ones_r, rhs=rec16,
                                     start=True, stop=True)
                    bc = npool.tile([128, QC], F32, tag="bc", name="bc")
                    nc.vector.tensor_copy(bc, pb)
                    cout = copool.tile([128, QC], F16, tag="cout", name="cout")
                    nc.vector.tensor_mul(cout, pc, bc)
                    nc.sync.dma_start(
                        out=ctx_loc[qc][h * D:(h + 1) * D, :], in_=cout)
                    if h == HPC - 1:
                        nc.gpsimd.collective_compute(
                            "AllGather",
                            mybir.AluOpType.bypass,
                            replica_groups=[list(range(NCORES))],
                            ins=[ctx_loc[qc].opt()],
                            outs=[ctx_ful[qc].opt()],
                        )
                return fire

            def outp_chunk(qc):
                cts = []
                for ti in range(NKIN):
                    ct = cpool.tile([128, QC], F16, tag=f"ct{ti}", name=f"ct{ti}")
                    nc.sync.dma_start(
                        out=ct, in_=ctx_ful[qc][ti * 128:(ti + 1) * 128, :])
                    cts.append(ct)
                for m in range(HPC):
                    po = pb_pool.tile([128, QC], F32, tag="pb", name="po")
                    for ti in range(NKIN):
                        nc.tensor.matmul(
                            po, lhsT=wo_sb[:, ti, m * D:(m + 1) * D],
                            rhs=cts[ti], start=(ti == 0), stop=(ti == NKIN - 1))
                    ot = opool.tile([128, QC], F32, tag="ot", name="ot")
                    nc.vector.tensor_copy(ot, po)
                    nc.sync.dma_start(
                        out=outT[m * 128:(m + 1) * 128, qc * QC:(qc + 1) * QC],
                        in_=ot)

            # big chunks first: their collectives complete while smaller
            # chunks compute; out-proj trails one chunk behind.
            for qc in range(NQ - 1, -1, -1):
                attn_chunk(qc)
                if qc + 1 < NQ:
                    outp_chunk(qc + 1)
            for fn in pend_norm:
                if fn is not None:
                    fn()
            pend_norm[0] = pend_norm[1] = None
            outp_chunk(0)

    _split_sync_waits(nc)
    return nc


_NC_CACHE = None


def _get_nc():
    global _NC_CACHE
    if _NC_CACHE is None:
        _NC_CACHE = build_nc()
    return _NC_CACHE


def _make_in_maps(x, cos, sin, Wq, Wk, Wv, Wo):
    f16 = np.float16
    xT = np.ascontiguousarray(x.reshape(S, DIN).T.astype(f16))
    cosT = np.ascontiguousarray(cos.T.astype(f16))
    sinT = np.ascontiguousarray(sin.T.astype(f16))
    in_maps = []
    for c in range(NCORES):
        g = c // 2
        in_maps.append({
            "xT": xT,
            "wqT": np.ascontiguousarray(Wq[c * 256:(c + 1) * 256, :].T.astype(f16)),
            "wkT": np.ascontiguousarray(Wk[g * 128:(g + 1) * 128, :].T.astype(f16)),
            "wvT": np.ascontiguousarray(Wv[g * 128:(g + 1) * 128, :].T.astype(f16)),
            "cosT": cosT,
            "sinT": sinT,
            "woT": np.ascontiguousarray(Wo[c * 256:(c + 1) * 256, :].T.astype(f16)),
        })
    return in_maps


def run(x, cos, sin, Wq, Wk, Wv, Wo, trace=False):
    nc = _get_nc()
    in_maps = _make_in_maps(x, cos, sin, Wq, Wk, Wv, Wo)
    res = run_bass_kernel_spmd(nc, in_maps, list(range(NCORES)), trace=trace)
    outT = np.concatenate([res.results[c]["outT"] for c in range(NCORES)], axis=0)
    out = np.ascontiguousarray(outT.T).reshape(1, S, DIN).astype(np.float32)
    return out, res


def kernel(x, mask, cos, sin, Wq, Wk, Wv, Wo):
    out, _ = run(np.asarray(x, dtype=np.float32), np.asarray(cos, np.float32),
                 np.asarray(sin, np.float32), np.asarray(Wq, np.float32),
                 np.asarray(Wk, np.float32), np.asarray(Wv, np.float32),
                 np.asarray(Wo, np.float32))
    return out


# revision 20
# speedup vs baseline: 2.5509x; 1.0035x over previous
"""GQA kernel for Trainium2, 8 NeuronCores.

Problem: x[1,4096,2048], H=16 heads, G=4 kv-groups, D=128, causal mask,
RoPE on q/k, out-proj. Sharding: 2 heads per core (core c -> heads 2c,2c+1,
kv-group c//2). All tensors fed pre-transposed so matmul contractions land
on SBUF partitions. fp16 on-chip (PSUM accumulation fp32); exp is computed
as exp(s/sqrt(D) - 6*ln2) so weights fit fp16; the 2^-6 scale cancels in
the softmax normalization.

Structure per core:
  proj:  QT/KT/V projections from xT (streamed once) + RoPE, all chunks.
  attn:  per-head causal attention in scoresT [k,q] orientation, q-chunks
         in DESCENDING order; per-chunk ctx written to a DRAM tile and
         AllGathered (8 small collectives, dependency-tracked, overlapped
         with later attention); softmax denom accumulated on DVE+GpSimd,
         reduced by ones-matmul, inverted by reciprocal_approx_fast,
         broadcast by PE matmul.
  outp:  column-parallel out-proj per gathered chunk, interleaved into the
         attention loop one chunk behind its collective.

Output per core: outT_c = out.T[c*256:(c+1)*256, :]; host concatenates and
transposes back.
"""

import sys

for _p in ("/opt/trn_rl_repo",):
    if _p not in sys.path:
        sys.path.append(_p)

from contextlib import ExitStack

import numpy as np

import concourse.bass as bass
import concourse.tile as tile
from concourse import masks, mybir
from concourse.bass_utils import run_bass_kernel_spmd

F32 = mybir.dt.float32
F32R = mybir.dt.float32r
F16 = mybir.dt.float16
S = 4096
MAX_WAITS = 1  # walrus CoreV3 rejects instructions with more sync waits


def _split_sync_waits(nc, maxw=MAX_WAITS):
    """Hoist excess sem waits onto NOPs inserted before the instruction on
    the same engine queue (queue order makes this equivalent)."""
    from concourse import mybir as mb
    n = 0
    for bassbb in nc.bb_map.values():
        bb = bassbb.bb
        insts = list(bb.instructions)
        out = []
        changed = False
        for ins in insts:
            si = ins.sync_info
            if si is not None and si.on_wait and len(si.on_wait) > maxw:
                waits = list(si.on_wait)
                head, rest = waits[:-maxw], waits[-maxw:]
                while head:
                    chunk, head = head[:maxw], head[maxw:]
                    n += 1
                    nop = mb.InstNoOp(
                        name=f"I-ws{n}",
                        engine=ins.engine,
                        ins=[],
                        outs=[],
                        sync_info=mb.SyncInfo(on_wait=chunk, on_update=[]),
                    )
                    nc.register_instruction(nop)
                    out.append(nop)
                ins.sync_info = mb.SyncInfo(
                    on_wait=rest, on_update=list(si.on_update or []))
                changed = True
            out.append(ins)
        if changed:
            try:
                bb.instructions[:] = out
            except TypeError:
                bb.set_instructions(out)
    return n


DIN = 2048
D = 128
HPC = 2          # heads per core
NCORES = 8
QC = 512         # q-chunk (free dim per matmul)
NQ = S // QC     # 8 q-chunks
KT = 128         # k tile (partition dim)
NKIN = DIN // 128  # 16 contraction tiles for projections
INV_SQRT_D = 1.0 / np.sqrt(D)
EXP_BIAS = float(-6.0 * np.log(2.0))  # 2^-6 scale on exp; cancels in softmax


def build_nc():
    nc = bass.Bass(num_devices=NCORES)

    xT = nc.dram_tensor("xT", [DIN, S], F16, kind="ExternalInput")
    wqT = nc.dram_tensor("wqT", [DIN, HPC * D], F16, kind="ExternalInput")
    # K weights on even cores, V weights on odd cores (pair-exchanged)
    wkvT = nc.dram_tensor("wkvT", [DIN, D], F16, kind="ExternalInput")
    cosT = nc.dram_tensor("cosT", [D, S], F16, kind="ExternalInput")
    sinT = nc.dram_tensor("sinT", [D, S], F16, kind="ExternalInput")
    woT = nc.dram_tensor("woT", [DIN, HPC * D], F16, kind="ExternalInput")
    outT = nc.dram_tensor("outT", [HPC * D, S], F32, kind="ExternalOutput")

    with ExitStack() as ctx:
        tc = ctx.enter_context(tile.TileContext(nc))

        res = ctx.enter_context(tc.tile_pool(name="res", bufs=1))
        dram = ctx.enter_context(tc.tile_pool(name="dram", bufs=1, space="DRAM"))

        # collective bounce tiles (dependency-tracked DRAM tiles)
        ctx_loc = [dram.tile([HPC * D, QC], F16, tag=f"cl{qc}", name=f"cl{qc}")
                   for qc in range(NQ)]
        ctx_ful = [dram.tile([NCORES * HPC * D, QC], F16, tag=f"cf{qc}",
                             name=f"cf{qc}")
                   for qc in range(NQ)]
        # K/V pair-exchange bounce tiles (core pair shares one kv-group;
        # even core projects K, odd core projects V, AllGather over pairs)
        kv_loc = [dram.tile([D, QC], F16, tag=f"kl{qc}", name=f"kl{qc}")
                  for qc in range(NQ)]
        kv_pair = [dram.tile([2 * D, QC], F16, tag=f"kp{qc}", name=f"kp{qc}")
                   for qc in range(NQ)]

        # resident SBUF tensors
        qt = res.tile([128, HPC, S], F16, tag="qt")          # QT per head
        kt = res.tile([128, S], F16, tag="kt")               # KT (shared group)
        vt = res.tile([128, S // 128, D], F16, tag="vt")     # V as s-tiles
        wq_sb = res.tile([128, NKIN, HPC * D], F16, tag="wq")
        wkv_sb = res.tile([128, NKIN, D], F16, tag="wkv")
        wo_sb = res.tile([128, NKIN, HPC * D], F16, tag="wo")
        cos_sb = res.tile([128, S], F16, tag="cos")
        sin_sb = res.tile([128, S], F16, tag="sin")
        ones_k = res.tile([128, 1], F16, tag="ones_k")       # lhsT for col sums
        ebias = res.tile([128, 1], F32, tag="ebias")         # exp bias 2^-6
        ones_r = res.tile([1, 128], F16, tag="ones_r")       # lhsT for bcast
        ident = res.tile([128, 128], F16, tag="ident")       # PE transpose id

        nc.vector.memset(ones_k, 1.0)
        nc.vector.memset(ebias, EXP_BIAS)
        nc.vector.memset(ones_r, 1.0)
        masks.make_identity(nc, ident)

        # weight loads
        nc.sync.dma_start(out=wq_sb, in_=wqT.rearrange("(t p) m -> p t m", p=128))
        nc.sync.dma_start(out=wkv_sb, in_=wkvT.rearrange("(t p) m -> p t m", p=128))
        nc.sync.dma_start(out=wo_sb, in_=woT.rearrange("(t p) m -> p t m", p=128))
        nc.sync.dma_start(out=cos_sb, in_=cosT[:, :])
        nc.sync.dma_start(out=sin_sb, in_=sinT[:, :])

        def rope(dst, src, rpool, cos_c, sin_c):
            # dst = src*cos + rot(src)*sin, rotate-half along partitions
            rot = rpool.tile([128, QC], F16, tag="rot")
            nc.vector.tensor_scalar_mul(rot[0:64, :], src[64:128, :], -1.0)
            nc.vector.tensor_copy(rot[64:128, :], src[0:64, :])
            nc.vector.tensor_mul(dst, src, cos_c)
            nc.vector.tensor_mul(rot, rot, sin_c)
            nc.vector.tensor_add(dst, dst, rot)

        # ---------------- projections + RoPE + K/V pair exchange ----------------
        with ExitStack() as p1:
            xpool = p1.enter_context(tc.tile_pool(name="xpool", bufs=4))
            rpool = p1.enter_context(tc.tile_pool(name="rope", bufs=3))
            kvpool = p1.enter_context(tc.tile_pool(name="kvp", bufs=2))
            pp_pool = p1.enter_context(tc.tile_pool(name="pp", bufs=2, space="PSUM"))
            pkv_pool = p1.enter_context(tc.tile_pool(name="pkv", bufs=2, space="PSUM"))
            pv2_pool = p1.enter_context(tc.tile_pool(name="pv2", bufs=1, space="PSUM"))

            def ingest_kv(qc):
                # read back the pair-gathered K/V chunk: rows 0:128 = KT
                # (even core's projection), rows 128:256 = VT (odd core's)
                q0 = qc * QC
                kb = kvpool.tile([128, QC], F16, tag="kb", name="kb")
                vb = kvpool.tile([128, QC], F16, tag="vb", name="vb")
                nc.sync.dma_start(out=kb, in_=kv_pair[qc][0:D, :])
                nc.sync.dma_start(out=vb, in_=kv_pair[qc][D:2 * D, :])
                rope(kt[:, q0:q0 + QC], kb, rpool,
                     cos_sb[:, q0:q0 + QC], sin_sb[:, q0:q0 + QC])
                pv2 = pv2_pool.tile([128, 4, D], F16, tag="pv2", name="pv2")
                for si in range(4):
                    nc.tensor.transpose(
                        pv2[:, si, :], vb[:, si * 128:(si + 1) * 128], ident)
                nc.vector.tensor_copy(vt[:, qc * 4:(qc + 1) * 4, :], pv2)

            for qc in range(NQ):
                q0 = qc * QC
                pq = pp_pool.tile([128, 2, QC], F32, tag="pp", name="pq")
                pkv = pkv_pool.tile([128, QC], F32, tag="pkv", name="pkv")
                for ki in range(NKIN):
                    xt = xpool.tile([128, QC], F16, tag="xt", name="xt")
                    nc.sync.dma_start(
                        out=xt, in_=xT[ki * 128:(ki + 1) * 128, q0:q0 + QC])
                    st = ki == 0
                    sp = ki == NKIN - 1
                    for h in range(HPC):
                        nc.tensor.matmul(
                            pq[:, h, :], lhsT=wq_sb[:, ki, h * D:(h + 1) * D],
                            rhs=xt, start=st, stop=sp)
                    nc.tensor.matmul(pkv, lhsT=wkv_sb[:, ki, :], rhs=xt,
                                     start=st, stop=sp)
                    if ki == 4 and qc > 0:
                        ingest_kv(qc - 1)
                kvout = kvpool.tile([128, QC], F16, tag="kvout", name="kvout")
                nc.vector.tensor_copy(kvout, pkv)
                nc.sync.dma_start(out=kv_loc[qc][:, :], in_=kvout)
                nc.gpsimd.collective_compute(
                    "AllGather",
                    mybir.AluOpType.bypass,
                    replica_groups=[[2 * p, 2 * p + 1]
                                    for p in range(NCORES // 2)],
                    ins=[kv_loc[qc].opt()],
                    outs=[kv_pair[qc].opt()],
                )
                for h in range(HPC):
                    rope(qt[:, h, q0:q0 + QC], pq[:, h, :], rpool,
                         cos_sb[:, q0:q0 + QC], sin_sb[:, q0:q0 + QC])
            ingest_kv(NQ - 1)

        # ---------------- attention (+ interleaved out-proj) ----------------
        with ExitStack() as p2:
            wpool = p2.enter_context(tc.tile_pool(name="wpool", bufs=4))
            apool = p2.enter_context(tc.tile_pool(name="acc", bufs=2))
            npool = p2.enter_context(tc.tile_pool(name="norm", bufs=2))
            copool = p2.enter_context(tc.tile_pool(name="cout", bufs=2))
            cpool = p2.enter_context(tc.tile_pool(name="cpool", bufs=2))
            opool = p2.enter_context(tc.tile_pool(name="opool", bufs=2))
            ps_pool = p2.enter_context(tc.tile_pool(name="ps", bufs=2, space="PSUM"))
            pc_pool = p2.enter_context(tc.tile_pool(name="pc", bufs=2, space="PSUM"))
            aux_pool = p2.enter_context(tc.tile_pool(name="aux", bufs=1, space="PSUM"))
            # one shared bank ring: out-proj accumulators and the softmax
            # broadcast both use it transiently
            pb_pool = p2.enter_context(tc.tile_pool(name="pb", bufs=1, space="PSUM"))

            # Pending normalize work, lagged one head so its serial
            # ACT chain hides behind the next head's matmuls.
            pend_norm = [None, None]  # [free_aux_fn, rest_fn]

            def attn_chunk(qc):
                q0 = qc * QC
                nk = (qc + 1) * 4
                ng = nk // 2
                for h in range(HPC):
                    pc = pc_pool.tile([128, QC], F32, tag="pc")
                    acc = apool.tile([128, QC], F16, tag="acc", name="acc")
                    acc2 = apool.tile([128, QC], F16, tag="acc2", name="acc2")
                    prev = None  # (wt, g) waiting for its pc matmuls

                    def consume(wt, g):
                        # pc matmuls for group g (lagged one group behind the
                        # score matmuls so the PE never waits on exp)
                        for j in range(2):
                            ki = 2 * g + j
                            nc.tensor.matmul(pc, lhsT=vt[:, ki, :],
                                             rhs=wt[:, j, :],
                                             start=(ki == 0), stop=(ki == nk - 1))

                    for g in range(ng):
                        ps = ps_pool.tile([128, 2, QC], F32, tag="ps")
                        wt = wpool.tile([128, 2, QC], F16, tag="wt")
                        for j in range(2):
                            k0 = (2 * g + j) * KT
                            nc.tensor.matmul(ps[:, j, :], lhsT=kt[:, k0:k0 + KT],
                                             rhs=qt[:, h, q0:q0 + QC],
                                             start=True, stop=True)
                        if g == 0 and pend_norm[0] is not None:
                            pend_norm[0]()  # prev head's denominator reduce
                            pend_norm[0] = None
                        if prev is not None:
                            consume(*prev)
                        if g == 1 and pend_norm[1] is not None:
                            pend_norm[1]()  # rest of prev head's normalize
                            pend_norm[1] = None
                        nc.scalar.activation(wt, ps,
                                             mybir.ActivationFunctionType.Exp,
                                             scale=INV_SQRT_D, bias=ebias[:, :])
                        for j in range(2):
                            k0 = (2 * g + j) * KT
                            if k0 + KT - 1 > q0:
                                # keep where (q0+col) - (k0+p) >= 0
                                nc.gpsimd.affine_select(
                                    out=wt[:, j, :], in_=wt[:, j, :],
                                    pattern=[[1, QC]],
                                    compare_op=mybir.AluOpType.is_ge, fill=0.0,
                                    base=q0 - k0, channel_multiplier=-1)
                        # softmax denominator partials, split across engines
                        if g == 0:
                            nc.vector.tensor_copy(acc, wt[:, 0, :])
                            nc.gpsimd.tensor_copy(acc2, wt[:, 1, :])
                        else:
                            nc.vector.tensor_add(acc, acc, wt[:, 0, :])
                            nc.gpsimd.tensor_add(acc2, acc2, wt[:, 1, :])
                        prev = (wt, g)
                    consume(*prev)
                    for fn in pend_norm:
                        if fn is not None:
                            fn()
                    pend_norm[0] = make_norm_a(qc, h, acc, acc2)
                    pend_norm[1] = make_norm_b(qc, h, pc)

            norm_state = {}

            def make_norm_a(qc, h, acc, acc2):
                def fire():
                    # cross-partition reduce of the exp sums on the PE
                    aux = aux_pool.tile([128, QC], F32, tag="aux", name="aux")
                    nc.tensor.matmul(aux[0:1, :], lhsT=ones_k, rhs=acc,
                                     start=True, stop=False)
                    nc.tensor.matmul(aux[0:1, :], lhsT=ones_k, rhs=acc2,
                                     start=False, stop=True)
                    den32 = npool.tile([1, QC], F32, tag="den32", name="den32")
                    nc.vector.tensor_copy(den32, aux[0:1, :])
                    norm_state[(qc, h)] = den32
                return fire

            def make_norm_b(qc, h, pc):
                def fire():
                    den32 = norm_state.pop((qc, h))
                    # 1/denom via exp(-ln(denom)) on the ACT engine
                    lden = npool.tile([1, QC], F32, tag="lden", name="lden")
                    nc.scalar.activation(lden, den32,
                                         mybir.ActivationFunctionType.Ln)
                    rec16 = npool.tile([1, QC], F16, tag="rec16", name="rec16")
                    nc.scalar.activation(rec16, lden,
                                         mybir.ActivationFunctionType.Exp,
                                         scale=-1.0)
                    pb = pb_pool.tile([128, QC], F32, tag="pb", name="pb")
                    nc.tensor.matmul(pb, lhsT=ones_r, rhs=rec16,
                                     start=True, stop=True)
                    bc = npool.tile([128, QC], F32, tag="bc", name="bc")
                    nc.vector.tensor_copy(bc, pb)
                    cout = copool.tile([128, QC], F16, tag="cout", name="cout")
                    nc.vector.tensor_mul(cout, pc, bc)
                    nc.sync.dma_start(
                        out=ctx_loc[qc][h * D:(h + 1) * D, :], in_=cout)
                    if h == HPC - 1:
                        nc.gpsimd.collective_compute(
                            "AllGather",
                            mybir.AluOpType.bypass,
                            replica_groups=[list(range(NCORES))],
                            ins=[ctx_loc[qc].opt()],
                            outs=[ctx_ful[qc].opt()],
                        )
                return fire

            def outp_chunk(qc):
                cts = []
                for ti in range(NKIN):
                    ct = cpool.tile([128, QC], F16, tag=f"ct{ti}", name=f"ct{ti}")
                    nc.sync.dma_start(
                        out=ct, in_=ctx_ful[qc][ti * 128:(ti + 1) * 128, :])
                    cts.append(ct)
                for m in range(HPC):
                    # m=0 uses the pb bank ring, m=1 the aux ring, so the two
                    # accumulations don't serialize on one bank
                    pool = pb_pool if m == 0 else aux_pool
                    po = pool.tile([128, QC], F32,
                                   tag="pb" if m == 0 else "aux", name="po")
                    for ti in range(NKIN):
                        nc.tensor.matmul(
                            po, lhsT=wo_sb[:, ti, m * D:(m + 1) * D],
                            rhs=cts[ti], start=(ti == 0), stop=(ti == NKIN - 1))
                    ot = opool.tile([128, QC], F32, tag="ot", name="ot")
                    nc.vector.tensor_copy(ot, po)
                    nc.sync.dma_start(
                        out=outT[m * 128:(m + 1) * 128, qc * QC:(qc + 1) * QC],
                        in_=ot)

            # big chunks first: their collectives complete while smaller
            # chunks compute; out-proj trails one chunk behind.
            for qc in range(NQ - 1, -1, -1):
                attn_chunk(qc)
                if qc + 1 < NQ:
                    outp_chunk(qc + 1)
            for fn in pend_norm:
                if fn is not None:
                    fn()
            pend_norm[0] = pend_norm[1] = None
            outp_chunk(0)

    _split_sync_waits(nc)
    return nc


_NC_CACHE = None


def _get_nc():
    global _NC_CACHE
    if _NC_CACHE is None:
        _NC_CACHE = build_nc()
    return _NC_CACHE


def _make_in_maps(x, cos, sin, Wq, Wk, Wv, Wo):
    f16 = np.float16
    xT = np.ascontiguousarray(x.reshape(S, DIN).T.astype(f16))
    cosT = np.ascontiguousarray(cos.T.astype(f16))
    sinT = np.ascontiguousarray(sin.T.astype(f16))
    in_maps = []
    for c in range(NCORES):
        g = c // 2
        wkv = Wk if c % 2 == 0 else Wv
        in_maps.append({
            "xT": xT,
            "wqT": np.ascontiguousarray(Wq[c * 256:(c + 1) * 256, :].T.astype(f16)),
            "wkvT": np.ascontiguousarray(
                wkv[g * 128:(g + 1) * 128, :].T.astype(f16)),
            "cosT": cosT,
            "sinT": sinT,
            "woT": np.ascontiguousarray(Wo[c * 256:(c + 1) * 256, :].T.astype(f16)),
        })
    return in_maps


def run(x, cos, sin, Wq, Wk, Wv, Wo, trace=False):
    nc = _get_nc()
    in_maps = _make_in_maps(x, cos, sin, Wq, Wk, Wv, Wo)
    res = run_bass_kernel_spmd(nc, in_maps, list(range(NCORES)), trace=trace)
    outT = np.concatenate([res.results[c]["outT"] for c in range(NCORES)], axis=0)
    out = np.ascontiguousarray(outT.T).reshape(1, S, DIN).astype(np.float32)
    return out, res


def kernel(x, mask, cos, sin, Wq, Wk, Wv, Wo):
    out, _ = run(np.asarray(x, dtype=np.float32), np.asarray(cos, np.float32),
                 np.asarray(sin, np.float32), np.asarray(Wq, np.float32),
                 np.asarray(Wk, np.float32), np.asarray(Wv, np.float32),
                 np.asarray(Wo, np.float32))
    return out


# revision 21
# speedup vs baseline: 2.6716x; 1.0473x over previous
"""GQA kernel for Trainium2, 8 NeuronCores.

Problem: x[1,4096,2048], H=16 heads, G=4 kv-groups, D=128, causal mask,
RoPE on q/k, out-proj. Sharding: 2 heads per core (core c -> heads 2c,2c+1,
kv-group c//2). All tensors fed pre-transposed so matmul contractions land
on SBUF partitions. fp16 on-chip (PSUM accumulation fp32); exp is computed
as exp(s/sqrt(D) - 6*ln2) so weights fit fp16; the 2^-6 scale cancels in
the softmax normalization.

Structure per core:
  proj:  QT/KT/V projections from xT (streamed once) + RoPE, all chunks.
  attn:  per-head causal attention in scoresT [k,q] orientation, q-chunks
         in DESCENDING order; per-chunk ctx written to a DRAM tile and
         AllGathered (8 small collectives, dependency-tracked, overlapped
         with later attention); softmax denom accumulated on DVE+GpSimd,
         reduced by ones-matmul, inverted by reciprocal_approx_fast,
         broadcast by PE matmul.
  outp:  column-parallel out-proj per gathered chunk, interleaved into the
         attention loop one chunk behind its collective.

Output per core: outT_c = out.T[c*256:(c+1)*256, :]; host concatenates and
transposes back.
"""

import sys

for _p in ("/opt/trn_rl_repo",):
    if _p not in sys.path:
        sys.path.append(_p)

from contextlib import ExitStack

import numpy as np

import concourse.bass as bass
import concourse.tile as tile
from concourse import masks, mybir
from concourse.bass_utils import run_bass_kernel_spmd

F32 = mybir.dt.float32
F32R = mybir.dt.float32r
F16 = mybir.dt.float16
S = 4096
MAX_WAITS = 1  # walrus CoreV3 rejects instructions with more sync waits


def _split_sync_waits(nc, maxw=MAX_WAITS):
    """Hoist excess sem waits onto NOPs inserted before the instruction on
    the same engine queue (queue order makes this equivalent)."""
    from concourse import mybir as mb
    n = 0
    for bassbb in nc.bb_map.values():
        bb = bassbb.bb
        insts = list(bb.instructions)
        out = []
        changed = False
        for ins in insts:
            si = ins.sync_info
            if si is not None and si.on_wait and len(si.on_wait) > maxw:
                waits = list(si.on_wait)
                head, rest = waits[:-maxw], waits[-maxw:]
                while head:
                    chunk, head = head[:maxw], head[maxw:]
                    n += 1
                    nop = mb.InstNoOp(
                        name=f"I-ws{n}",
                        engine=ins.engine,
                        ins=[],
                        outs=[],
                        sync_info=mb.SyncInfo(on_wait=chunk, on_update=[]),
                    )
                    nc.register_instruction(nop)
                    out.append(nop)
                ins.sync_info = mb.SyncInfo(
                    on_wait=rest, on_update=list(si.on_update or []))
                changed = True
            out.append(ins)
        if changed:
            try:
                bb.instructions[:] = out
            except TypeError:
                bb.set_instructions(out)
    return n


DIN = 2048
D = 128
HPC = 2          # heads per core
NCORES = 8
QC = 512         # q-chunk (free dim per matmul)
NQ = S // QC     # 8 q-chunks
KT = 128         # k tile (partition dim)
NKIN = DIN // 128  # 16 contraction tiles for projections
INV_SQRT_D = 1.0 / np.sqrt(D)
EXP_BIAS = float(-6.0 * np.log(2.0))  # 2^-6 scale on exp; cancels in softmax


def build_nc():
    nc = bass.Bass(num_devices=NCORES)

    xT = nc.dram_tensor("xT", [DIN, S], F16, kind="ExternalInput")
    wqT = nc.dram_tensor("wqT", [DIN, HPC * D], F16, kind="ExternalInput")
    # K weights on even cores, V weights on odd cores (pair-exchanged)
    wkvT = nc.dram_tensor("wkvT", [DIN, D], F16, kind="ExternalInput")
    cosT = nc.dram_tensor("cosT", [D, S], F16, kind="ExternalInput")
    sinT = nc.dram_tensor("sinT", [D, S], F16, kind="ExternalInput")
    woT = nc.dram_tensor("woT", [DIN, HPC * D], F16, kind="ExternalInput")
    outT = nc.dram_tensor("outT", [HPC * D, S], F32, kind="ExternalOutput")

    with ExitStack() as ctx:
        tc = ctx.enter_context(tile.TileContext(nc))

        res = ctx.enter_context(tc.tile_pool(name="res", bufs=1))
        dram = ctx.enter_context(tc.tile_pool(name="dram", bufs=1, space="DRAM"))

        # collective bounce tiles (dependency-tracked DRAM tiles)
        ctx_loc = [dram.tile([HPC * D, QC], F16, tag=f"cl{qc}", name=f"cl{qc}")
                   for qc in range(NQ)]
        ctx_ful = [dram.tile([NCORES * HPC * D, QC], F16, tag=f"cf{qc}",
                             name=f"cf{qc}")
                   for qc in range(NQ)]
        # K/V pair-exchange bounce tiles (core pair shares one kv-group;
        # even core projects K, odd core projects V, AllGather over pairs;
        # two q-chunks merged per collective to amortize rendezvous latency)
        kv_loc = [dram.tile([D, 2 * QC], F16, tag=f"kl{p}", name=f"kl{p}")
                  for p in range(NQ // 2)]
        kv_pair = [dram.tile([2 * D, 2 * QC], F16, tag=f"kp{p}", name=f"kp{p}")
                   for p in range(NQ // 2)]

        # resident SBUF tensors
        qt = res.tile([128, HPC, S], F16, tag="qt")          # QT per head
        kt = res.tile([128, S], F16, tag="kt")               # KT (shared group)
        vt = res.tile([128, S // 128, D], F16, tag="vt")     # V as s-tiles
        wq_sb = res.tile([128, NKIN, HPC * D], F16, tag="wq")
        wkv_sb = res.tile([128, NKIN, D], F16, tag="wkv")
        wo_sb = res.tile([128, NKIN, HPC * D], F16, tag="wo")
        cos_sb = res.tile([128, S], F16, tag="cos")
        sin_sb = res.tile([128, S], F16, tag="sin")
        ones_k = res.tile([128, 1], F16, tag="ones_k")       # lhsT for col sums
        ebias = res.tile([128, 1], F32, tag="ebias")         # exp bias 2^-6
        ones_r = res.tile([1, 128], F16, tag="ones_r")       # lhsT for bcast
        ident = res.tile([128, 128], F16, tag="ident")       # PE transpose id

        nc.vector.memset(ones_k, 1.0)
        nc.vector.memset(ebias, EXP_BIAS)
        nc.vector.memset(ones_r, 1.0)
        masks.make_identity(nc, ident)

        # weight loads
        nc.sync.dma_start(out=wq_sb, in_=wqT.rearrange("(t p) m -> p t m", p=128))
        nc.sync.dma_start(out=wkv_sb, in_=wkvT.rearrange("(t p) m -> p t m", p=128))
        nc.sync.dma_start(out=wo_sb, in_=woT.rearrange("(t p) m -> p t m", p=128))
        nc.sync.dma_start(out=cos_sb, in_=cosT[:, :])
        nc.sync.dma_start(out=sin_sb, in_=sinT[:, :])

        def rope(dst, src, rpool, cos_c, sin_c):
            # dst = src*cos + rot(src)*sin, rotate-half along partitions
            rot = rpool.tile([128, QC], F16, tag="rot")
            nc.vector.tensor_scalar_mul(rot[0:64, :], src[64:128, :], -1.0)
            nc.vector.tensor_copy(rot[64:128, :], src[0:64, :])
            nc.vector.tensor_mul(dst, src, cos_c)
            nc.vector.tensor_mul(rot, rot, sin_c)
            nc.vector.tensor_add(dst, dst, rot)

        # ---------------- projections + RoPE + K/V pair exchange ----------------
        with ExitStack() as p1:
            xpool = p1.enter_context(tc.tile_pool(name="xpool", bufs=4))
            rpool = p1.enter_context(tc.tile_pool(name="rope", bufs=3))
            kvpool = p1.enter_context(tc.tile_pool(name="kvp", bufs=2))
            pp_pool = p1.enter_context(tc.tile_pool(name="pp", bufs=2, space="PSUM"))
            pkv_pool = p1.enter_context(tc.tile_pool(name="pkv", bufs=2, space="PSUM"))
            pv2_pool = p1.enter_context(tc.tile_pool(name="pv2", bufs=1, space="PSUM"))

            def ingest_kv(qc):
                # read back the pair-gathered K/V chunk: rows 0:128 = KT
                # (even core's projection), rows 128:256 = VT (odd core's)
                q0 = qc * QC
                p, half = qc // 2, (qc % 2) * QC
                kb = kvpool.tile([128, QC], F16, tag="kb", name="kb")
                vb = kvpool.tile([128, QC], F16, tag="vb", name="vb")
                nc.sync.dma_start(out=kb, in_=kv_pair[p][0:D, half:half + QC])
                nc.sync.dma_start(out=vb,
                                  in_=kv_pair[p][D:2 * D, half:half + QC])
                rope(kt[:, q0:q0 + QC], kb, rpool,
                     cos_sb[:, q0:q0 + QC], sin_sb[:, q0:q0 + QC])
                pv2 = pv2_pool.tile([128, 4, D], F16, tag="pv2", name="pv2")
                for si in range(4):
                    nc.tensor.transpose(
                        pv2[:, si, :], vb[:, si * 128:(si + 1) * 128], ident)
                nc.vector.tensor_copy(vt[:, qc * 4:(qc + 1) * 4, :], pv2)

            for qc in range(NQ):
                q0 = qc * QC
                pq = pp_pool.tile([128, 2, QC], F32, tag="pp", name="pq")
                pkv = pkv_pool.tile([128, QC], F32, tag="pkv", name="pkv")
                for ki in range(NKIN):
                    xt = xpool.tile([128, QC], F16, tag="xt", name="xt")
                    nc.sync.dma_start(
                        out=xt, in_=xT[ki * 128:(ki + 1) * 128, q0:q0 + QC])
                    st = ki == 0
                    sp = ki == NKIN - 1
                    for h in range(HPC):
                        nc.tensor.matmul(
                            pq[:, h, :], lhsT=wq_sb[:, ki, h * D:(h + 1) * D],
                            rhs=xt, start=st, stop=sp)
                    nc.tensor.matmul(pkv, lhsT=wkv_sb[:, ki, :], rhs=xt,
                                     start=st, stop=sp)
                    if ki == 4 and qc >= 2:
                        ingest_kv(qc - 2)
                kvout = kvpool.tile([128, QC], F16, tag="kvout", name="kvout")
                nc.vector.tensor_copy(kvout, pkv)
                half = (qc % 2) * QC
                nc.sync.dma_start(out=kv_loc[qc // 2][:, half:half + QC],
                                  in_=kvout)
                if qc % 2 == 1:
                    nc.gpsimd.collective_compute(
                        "AllGather",
                        mybir.AluOpType.bypass,
                        replica_groups=[[2 * p, 2 * p + 1]
                                        for p in range(NCORES // 2)],
                        ins=[kv_loc[qc // 2].opt()],
                        outs=[kv_pair[qc // 2].opt()],
                    )
                for h in range(HPC):
                    rope(qt[:, h, q0:q0 + QC], pq[:, h, :], rpool,
                         cos_sb[:, q0:q0 + QC], sin_sb[:, q0:q0 + QC])
            ingest_kv(NQ - 2)
            ingest_kv(NQ - 1)

        # ---------------- attention (+ interleaved out-proj) ----------------
        with ExitStack() as p2:
            wpool = p2.enter_context(tc.tile_pool(name="wpool", bufs=4))
            apool = p2.enter_context(tc.tile_pool(name="acc", bufs=2))
            npool = p2.enter_context(tc.tile_pool(name="norm", bufs=2))
            copool = p2.enter_context(tc.tile_pool(name="cout", bufs=2))
            cpool = p2.enter_context(tc.tile_pool(name="cpool", bufs=2))
            opool = p2.enter_context(tc.tile_pool(name="opool", bufs=2))
            ps_pool = p2.enter_context(tc.tile_pool(name="ps", bufs=2, space="PSUM"))
            pc_pool = p2.enter_context(tc.tile_pool(name="pc", bufs=2, space="PSUM"))
            aux_pool = p2.enter_context(tc.tile_pool(name="aux", bufs=1, space="PSUM"))
            # one shared bank ring: out-proj accumulators and the softmax
            # broadcast both use it transiently
            pb_pool = p2.enter_context(tc.tile_pool(name="pb", bufs=1, space="PSUM"))

            # Pending normalize work, lagged one head so its serial
            # ACT chain hides behind the next head's matmuls.
            pend_norm = [None, None]  # [free_aux_fn, rest_fn]

            def attn_chunk(qc):
                q0 = qc * QC
                nk = (qc + 1) * 4
                ng = nk // 2
                for h in range(HPC):
                    pc = pc_pool.tile([128, QC], F32, tag="pc")
                    acc = apool.tile([128, QC], F16, tag="acc", name="acc")
                    acc2 = apool.tile([128, QC], F16, tag="acc2", name="acc2")
                    prev = None  # (wt, g) waiting for its pc matmuls

                    def consume(wt, g):
                        # pc matmuls for group g (lagged one group behind the
                        # score matmuls so the PE never waits on exp)
                        for j in range(2):
                            ki = 2 * g + j
                            nc.tensor.matmul(pc, lhsT=vt[:, ki, :],
                                             rhs=wt[:, j, :],
                                             start=(ki == 0), stop=(ki == nk - 1))

                    for g in range(ng):
                        ps = ps_pool.tile([128, 2, QC], F32, tag="ps")
                        wt = wpool.tile([128, 2, QC], F16, tag="wt")
                        for j in range(2):
                            k0 = (2 * g + j) * KT
                            nc.tensor.matmul(ps[:, j, :], lhsT=kt[:, k0:k0 + KT],
                                             rhs=qt[:, h, q0:q0 + QC],
                                             start=True, stop=True)
                        if g == 0 and pend_norm[0] is not None:
                            pend_norm[0]()  # prev head's denominator reduce
                            pend_norm[0] = None
                        if prev is not None:
                            consume(*prev)
                        if g == 1 and pend_norm[1] is not None:
                            pend_norm[1]()  # rest of prev head's normalize
                            pend_norm[1] = None
                        nc.scalar.activation(wt, ps,
                                             mybir.ActivationFunctionType.Exp,
                                             scale=INV_SQRT_D, bias=ebias[:, :])
                        for j in range(2):
                            k0 = (2 * g + j) * KT
                            if k0 + KT - 1 > q0:
                                # keep where (q0+col) - (k0+p) >= 0
                                nc.gpsimd.affine_select(
                                    out=wt[:, j, :], in_=wt[:, j, :],
                                    pattern=[[1, QC]],
                                    compare_op=mybir.AluOpType.is_ge, fill=0.0,
                                    base=q0 - k0, channel_multiplier=-1)
                        # softmax denominator partials, split across engines
                        if g == 0:
                            nc.vector.tensor_copy(acc, wt[:, 0, :])
                            nc.gpsimd.tensor_copy(acc2, wt[:, 1, :])
                        else:
                            nc.vector.tensor_add(acc, acc, wt[:, 0, :])
                            nc.gpsimd.tensor_add(acc2, acc2, wt[:, 1, :])
                        prev = (wt, g)
                    consume(*prev)
                    for fn in pend_norm:
                        if fn is not None:
                            fn()
                    pend_norm[0] = make_norm_a(qc, h, acc, acc2)
                    pend_norm[1] = make_norm_b(qc, h, pc)

            norm_state = {}

            def make_norm_a(qc, h, acc, acc2):
                def fire():
                    # cross-partition reduce of the exp sums on the PE
                    aux = aux_pool.tile([128, QC], F32, tag="aux", name="aux")
                    nc.tensor.matmul(aux[0:1, :], lhsT=ones_k, rhs=acc,
                                     start=True, stop=False)
                    nc.tensor.matmul(aux[0:1, :], lhsT=ones_k, rhs=acc2,
                                     start=False, stop=True)
                    den32 = npool.tile([1, QC], F32, tag="den32", name="den32")
                    nc.vector.tensor_copy(den32, aux[0:1, :])
                    norm_state[(qc, h)] = den32
                return fire

            def make_norm_b(qc, h, pc):
                def fire():
                    den32 = norm_state.pop((qc, h))
                    # 1/denom via exp(-ln(denom)) on the ACT engine
                    lden = npool.tile([1, QC], F32, tag="lden", name="lden")
                    nc.scalar.activation(lden, den32,
                                         mybir.ActivationFunctionType.Ln)
                    rec16 = npool.tile([1, QC], F16, tag="rec16", name="rec16")
                    nc.scalar.activation(rec16, lden,
                                         mybir.ActivationFunctionType.Exp,
                                         scale=-1.0)
                    pb = pb_pool.tile([128, QC], F32, tag="pb", name="pb")
                    nc.tensor.matmul(pb, lhsT=ones_r, rhs=rec16,
                                     start=True, stop=True)
                    bc = npool.tile([128, QC], F32, tag="bc", name="bc")
                    nc.vector.tensor_copy(bc, pb)
                    cout = copool.tile([128, QC], F16, tag="cout", name="cout")
                    nc.vector.tensor_mul(cout, pc, bc)
                    nc.sync.dma_start(
                        out=ctx_loc[qc][h * D:(h + 1) * D, :], in_=cout)
                    if h == HPC - 1:
                        nc.gpsimd.collective_compute(
                            "AllGather",
                            mybir.AluOpType.bypass,
                            replica_groups=[list(range(NCORES))],
                            ins=[ctx_loc[qc].opt()],
                            outs=[ctx_ful[qc].opt()],
                        )
                return fire

            def outp_chunk(qc):
                cts = []
                for ti in range(NKIN):
                    ct = cpool.tile([128, QC], F16, tag=f"ct{ti}", name=f"ct{ti}")
                    nc.sync.dma_start(
                        out=ct, in_=ctx_ful[qc][ti * 128:(ti + 1) * 128, :])
                    cts.append(ct)
                for m in range(HPC):
                    # m=0 uses the pb bank ring, m=1 the aux ring, so the two
                    # accumulations don't serialize on one bank
                    pool = pb_pool if m == 0 else aux_pool
                    po = pool.tile([128, QC], F32,
                                   tag="pb" if m == 0 else "aux", name="po")
                    for ti in range(NKIN):
                        nc.tensor.matmul(
                            po, lhsT=wo_sb[:, ti, m * D:(m + 1) * D],
                            rhs=cts[ti], start=(ti == 0), stop=(ti == NKIN - 1))
                    ot = opool.tile([128, QC], F32, tag="ot", name="ot")
                    nc.vector.tensor_copy(ot, po)
                    nc.sync.dma_start(
                        out=outT[m * 128:(m + 1) * 128, qc * QC:(qc + 1) * QC],
                        in_=ot)

            # small/big chunk interleave: each chunk's ctx collective hides
            # under the next (larger) chunk's attention; out-proj trails one
            # position behind.
            order = [3, 7, 2, 6, 1, 5, 0, 4]
            for i, qc in enumerate(order):
                attn_chunk(qc)
                if i > 0:
                    outp_chunk(order[i - 1])
            for fn in pend_norm:
                if fn is not None:
                    fn()
            pend_norm[0] = pend_norm[1] = None
            outp_chunk(order[-1])

    _split_sync_waits(nc)
    return nc


_NC_CACHE = None


def _get_nc():
    global _NC_CACHE
    if _NC_CACHE is None:
        _NC_CACHE = build_nc()
    return _NC_CACHE


def _make_in_maps(x, cos, sin, Wq, Wk, Wv, Wo):
    f16 = np.float16
    xT = np.ascontiguousarray(x.reshape(S, DIN).T.astype(f16))
    cosT = np.ascontiguousarray(cos.T.astype(f16))
    sinT = np.ascontiguousarray(sin.T.astype(f16))
    in_maps = []
    for c in range(NCORES):
        g = c // 2
        wkv = Wk if c % 2 == 0 else Wv
        in_maps.append({
            "xT": xT,
            "wqT": np.ascontiguousarray(Wq[c * 256:(c + 1) * 256, :].T.astype(f16)),
            "wkvT": np.ascontiguousarray(
                wkv[g * 128:(g + 1) * 128, :].T.astype(f16)),
            "cosT": cosT,
            "sinT": sinT,
            "woT": np.ascontiguousarray(Wo[c * 256:(c + 1) * 256, :].T.astype(f16)),
        })
    return in_maps


def run(x, cos, sin, Wq, Wk, Wv, Wo, trace=False):
    nc = _get_nc()
    in_maps = _make_in_maps(x, cos, sin, Wq, Wk, Wv, Wo)
    res = run_bass_kernel_spmd(nc, in_maps, list(range(NCORES)), trace=trace)
    outT = np.concatenate([res.results[c]["outT"] for c in range(NCORES)], axis=0)
    out = np.ascontiguousarray(outT.T).reshape(1, S, DIN).astype(np.float32)
    return out, res


def kernel(x, mask, cos, sin, Wq, Wk, Wv, Wo):
    out, _ = run(np.asarray(x, dtype=np.float32), np.asarray(cos, np.float32),
                 np.asarray(sin, np.float32), np.asarray(Wq, np.float32),
                 np.asarray(Wk, np.float32), np.asarray(Wv, np.float32),
                 np.asarray(Wo, np.float32))
    return out
